# revision 51
# baseline (speedup 1.0000x reference)
"""Trainium2 Bass kernel for GQA attention (dense_transformer).

Sharding: 8 cores = 2-way data parallel (batch) x 4-way tensor parallel (heads).
Core c handles batch b=c//4, head group g=c%4 (8 q heads, 2 kv heads).
Per core: QKV projections (bf16, f32 accum), RoPE, causal attention with
transposed-scores layout (scoresT[k,q] -> probsT used directly as the moving
operand of the PV matmul; no on-chip transposes), per-chunk AllGather of
attention outputs within each 4-core group, then a column-sharded Wo matmul.
Host assembles disjoint output shards (pure unshard, no host math).

All inputs are host-packed into the exact SBUF tile layouts so every DMA is a
plain 2D transfer (contiguous per partition).
"""
import sys

sys.path.insert(0, "/opt/trn_rl_repo")

import numpy as np
import ml_dtypes

import concourse.bacc as bacc
import concourse.mybir as mybir
import concourse.tile as tile
from concourse import bass_isa
from concourse import bass_utils

BF16 = ml_dtypes.bfloat16

B, S, HID = 2, 2048, 4096
NH, NKV, D = 32, 8, 128
NCORES, GRP = 8, 4          # 2 groups of 4 cores
QH, KVH = NH // GRP, NKV // GRP   # 8 q heads, 2 kv heads per core
QD, KVD = QH * D, KVH * D         # 1024, 256
CH, NCH = 512, S // 512           # q-chunk size / count
KB = 128                          # k block
MW = CH // 2                      # masked-block q width (diag refinement)
NIT = HID // 128                  # 32 contraction tiles
SM_SCALE = float(D) ** -0.5
REPLICA_GROUPS = [[0, 1, 2, 3], [4, 5, 6, 7]]

_CACHE: dict = {}


# ---------------------------------------------------------------- builder
def build_nc(plan, nmask, debug_dumps=False):
    """plan[c] = list of (t, mask_idx_or_None) k-blocks for q-chunk c."""
    fp32, bf16, f32r = mybir.dt.float32, mybir.dt.bfloat16, mybir.dt.float32r
    nc = bacc.Bacc("TRN2", target_bir_lowering=False, debug=False,
                   num_devices=NCORES)

    # host-packed inputs (exact SBUF layouts; all DMAs contiguous/partition)
    xT = nc.dram_tensor("xT", [NCH, 128, NIT * CH], bf16, kind="ExternalInput")
    wq = nc.dram_tensor("wq", [QH, 128, NIT * 128], bf16, kind="ExternalInput")
    wk = nc.dram_tensor("wk", [128, NIT * KVD], bf16, kind="ExternalInput")
    wv = nc.dram_tensor("wv", [128, NIT * KVD], bf16, kind="ExternalInput")
    wo = nc.dram_tensor("wo", [QH, 128, NIT * 128], bf16, kind="ExternalInput")
    cosT = nc.dram_tensor("cosT", [D, S], bf16, kind="ExternalInput")
    sinS = nc.dram_tensor("sinS", [D, S], bf16, kind="ExternalInput")
    nm = max(nmask, 1)
    dmask = nc.dram_tensor("dmask", [KB, nm * MW], bf16, kind="ExternalInput")
    outT = nc.dram_tensor("outT", [QD, S], fp32, kind="ExternalOutput")

    # partition-major bounce buffers, one per head-quarter (2 heads): each
    # gather issues once its 2 heads are flushed; finer granularity loses to
    # per-op CC overhead, coarser exposes latency at chunk boundaries.
    # Gather concatenates the 4 group members along dim0 -> [GRP*128, 2*CH]
    NQD = QH // 2
    bnc_in = [[nc.dram_tensor(f"bnc_in{c}_{g}", [128, 2 * CH], bf16)
               for g in range(NQD)] for c in range(NCH)]
    bnc_out = [[nc.dram_tensor(f"bnc_out{c}_{g}", [GRP * 128, 2 * CH], bf16)
                for g in range(NQD)] for c in range(NCH)]

    # tiny buffers for a warm-up AllGather that absorbs the CC pipeline's
    # cold-start / peer-rendezvous cost (~12us) during proj(0)
    warm_in = nc.dram_tensor("warm_in", [128, 8], bf16)
    warm_out = nc.dram_tensor("warm_out", [GRP * 128, 8], bf16)
    import os as _os
    _salt = _os.environ.get("KSALT", "")
    if _salt:
        nc.dram_tensor(f"salt{_salt}", [1, 8], bf16)

    dbg = {}
    if debug_dumps:
        dbg["qt0"] = nc.dram_tensor("dbg_qt0", [128, QH * CH], bf16,
                                    kind="ExternalOutput")
        dbg["kt0"] = nc.dram_tensor("dbg_kt0", [D, S], bf16,
                                    kind="ExternalOutput")
        dbg["v"] = nc.dram_tensor("dbg_v", [128, (S // 128) * KVD], bf16,
                                  kind="ExternalOutput")

    with tile.TileContext(nc) as tc:
        from contextlib import ExitStack
        with ExitStack() as ctx:
            P = lambda **kw: ctx.enter_context(tc.tile_pool(**kw))
            const_p = P(name="const", bufs=1)
            wkv_p = P(name="wkv", bufs=1)
            res_p = P(name="res", bufs=1)         # kT, v, cos, sin, masks
            xt_p = P(name="xt", bufs=4)
            wq_p = P(name="wqp", bufs=3)
            qt_p = P(name="qt", bufs=2)
            rope_p = P(name="rope", bufs=2)
            probs_p = P(name="probs", bufs=3)
            attn_p = P(name="attn", bufs=3)
            gath_p = P(name="gath", bufs=1)
            wo_p = P(name="wop", bufs=2)
            out_p = P(name="outp", bufs=2)
            recip_p = P(name="recip", bufs=2)
            pA = P(name="pA", bufs=2, space="PSUM")
            psc = P(name="psc", bufs=4, space="PSUM")
            po = P(name="po", bufs=2, space="PSUM")

            # x^T chunk loads: 4 quarter tiles (8 i-tiles each). Chunk 0 goes
            # on the scalar HWDGE ring (fast, empty at t=0) so Q-proj can
            # start ~4us in; later chunks prefetch on the idle gpsimd ring,
            # one quarter after each gather trigger of the previous attn.
            NXQ = 4
            XQW = (NIT // NXQ) * CH   # free width of one quarter

            def alloc_xt(c):
                return [(xt_p.tile([128, XQW], bf16, tag="xt",
                                   name=f"xt{c}_{q}"), c, q)
                        for q in range(NXQ)]

            def issue_xt(entry, eng):
                t, c, q = entry
                eng.dma_start(t[:], xT.ap()[c][:, q * XQW:(q + 1) * XQW])

            def xt_sl(xtq, i, lo, hi):
                """slice of contraction tile i, columns [lo:hi) of the chunk"""
                q, r = divmod(i, NIT // NXQ)
                return xtq[q][0][:, r * CH + lo:r * CH + hi]

            xtq0 = alloc_xt(0)
            for e in xtq0:
                issue_xt(e, nc.scalar)

            # resident loads (all straight 2D); cos/sin first (needed by the
            # first rope ~12us in), then wk/wv (k-proj ~70us in), dmask last
            cos_sb = res_p.tile([D, S], bf16, tag="cos")
            nc.scalar.dma_start(cos_sb[:], cosT.ap())
            sin_sb = res_p.tile([D, S], bf16, tag="sin")
            nc.scalar.dma_start(sin_sb[:], sinS.ap())
            wk_sb = wkv_p.tile([128, NIT * KVD], bf16, tag="wk")
            nc.scalar.dma_start(wk_sb[:], wk.ap())
            wv_sb = wkv_p.tile([128, NIT * KVD], bf16, tag="wv")
            nc.scalar.dma_start(wv_sb[:], wv.ap())
            dm_sb = res_p.tile([KB, nm * MW], bf16, tag="dm")
            nc.scalar.dma_start(dm_sb[:], dmask.ap())
            kT_sb = [res_p.tile([D, S], bf16, tag=f"kT{kj}", name=f"kT{kj}")
                     for kj in range(KVH)]
            v_sb = res_p.tile([128, (S // 128) * KVD], bf16, tag="v")

            # warm up the collective pipeline (cold-start is ~12us) so chunk
            # 0's real gathers start promptly
            nc.gpsimd.collective_compute(
                "AllGather", mybir.AluOpType.bypass,
                replica_groups=REPLICA_GROUPS,
                ins=[warm_in.ap().opt()], outs=[warm_out.ap().opt()])

            Exp = mybir.ActivationFunctionType.Exp

            def rope(dst, ps, c):
                """dst (bf16 [128,CH]) = rope(ps) with cos/sin chunk c."""
                cs = cos_sb[:, c * CH:(c + 1) * CH]
                sn = sin_sb[:, c * CH:(c + 1) * CH]
                m1 = rope_p.tile([128, CH], fp32, tag="m1")
                m2 = rope_p.tile([128, CH], fp32, tag="m2")
                nc.vector.tensor_mul(m1[:], ps[:], cs)
                nc.vector.tensor_mul(m2[0:64, :], ps[64:128, :], sn[0:64, :])
                nc.vector.tensor_mul(m2[64:128, :], ps[0:64, :], sn[64:128, :])
                nc.vector.tensor_add(dst, m1[:], m2[:])

            def proj(c, qt, xtq):
                for j in range(QH):
                    wq_t = wq_p.tile([128, NIT * 128], bf16, tag="wq")
                    if c == 0 and j == 0:
                        # split the cold-start-critical first weight tile so
                        # the very first matmuls begin after ~256KB, not 1MB
                        W4 = (NIT // 4) * 128
                        for q4 in range(4):
                            nc.sync.dma_start(
                                wq_t[:, q4 * W4:(q4 + 1) * W4],
                                wq.ap()[j][:, q4 * W4:(q4 + 1) * W4])
                    else:
                        nc.sync.dma_start(wq_t[:], wq.ap()[j])
                    ps = pA.tile([128, CH], fp32, tag="pA")
                    for i in range(NIT):
                        nc.tensor.matmul(
                            ps[:], wq_t[:, i * 128:(i + 1) * 128],
                            xt_sl(xtq, i, 0, CH),
                            start=(i == 0), stop=(i == NIT - 1))
                    rope(qt[:, j * CH:(j + 1) * CH], ps, c)
                for kj in range(KVH):
                    ps = pA.tile([128, CH], fp32, tag="pA")
                    for i in range(NIT):
                        nc.tensor.matmul(
                            ps[:], wk_sb[:, i * KVD + kj * 128:
                                         i * KVD + (kj + 1) * 128],
                            xt_sl(xtq, i, 0, CH),
                            start=(i == 0), stop=(i == NIT - 1))
                    rope(kT_sb[kj][:, c * CH:(c + 1) * CH], ps, c)
                for t in range(CH // 128):
                    ps = pA.tile([128, KVD], fp32, tag="pA")
                    for i in range(NIT):
                        nc.tensor.matmul(
                            ps[:], xt_sl(xtq, i, t * 128, (t + 1) * 128),
                            wv_sb[:, i * KVD:(i + 1) * KVD],
                            start=(i == 0), stop=(i == NIT - 1))
                    sb = (c * (CH // 128) + t) * KVD
                    nc.scalar.copy(v_sb[:, sb:sb + KVD], ps[:])

            # deferred normalization tail: at head h's end the prob-sum is
            # reduced across partitions on the idle gpsimd engine (output
            # replicated to all partitions); the reciprocal + multiply + DMA
            # (flush) run while head h+1's scores are already streaming on PE.
            def make_tail(c, h, po_t, acc):
                bc = recip_p.tile([128, CH], fp32, tag="bc", name=f"bc{c}_{h}",
                                  bufs=1)
                nc.gpsimd.partition_all_reduce(bc[:], acc[:], 128,
                                               bass_isa.ReduceOp.add)

                def flush():
                    rc = recip_p.tile([128, CH], fp32, tag="rcf",
                                      name=f"rc{c}_{h}", bufs=1)
                    nc.vector.reciprocal_approx_fast(rc[:], bc[:])
                    at = attn_p.tile([128, CH], bf16, tag="at")
                    nc.vector.tensor_mul(at[:], po_t[:], rc[:])
                    nc.scalar.dma_start(
                        bnc_in[c][h // 2].ap()[:, (h % 2) * CH:
                                               (h % 2 + 1) * CH], at[:])
                return flush

            def wo_steps(c, wpool=None, wtag="wo", gpool=None, gtagf=None,
                         geng=None):
                """Generator of emission closures for the Wo phase of chunk c
                (interleaved into the next chunk's attention to fill PE
                bubbles left by the exp chain). Contraction runs quarter-major
                so quarter-gather g gates only tiles 8g..8g+7 of each chain —
                the final chunk's last gather overlaps 3/4 of its Wo work.
                The final chunk passes retired pools (qt for weights, xt for
                gathers) and the scalar ring for gathers so its DMAs never
                queue behind the leftover drain's weight loads."""
                wpool = wpool or wo_p
                gpool = gpool or gath_p
                gtagf = gtagf or (lambda g: f"gt{g}")
                geng = geng or nc.sync
                gts = [None] * NQD

                def ensure_gt(g):
                    # lazily emitted right before the first consuming matmul:
                    # a not-yet-done collective then only delays weight
                    # prefetches, never the ACT exp stream
                    if gts[g] is None:
                        gt = gpool.tile([128, GRP * 2 * CH], bf16,
                                        tag=gtagf(g), name=f"gt{c}_{g}")
                        geng.dma_start(
                            gt[:].rearrange("p (g f) -> p g f", g=GRP),
                            bnc_out[c][g].ap().rearrange("(g p) f -> p g f",
                                                         p=128))
                        gts[g] = gt
                # hid tile H = cg*QH + h lives in quarter-gather h//2 at
                # column (cg*2 + h%2)*CH of that gather tile
                order = [(h // 2, cg * QH + h) for h in range(QH)
                         for cg in range(GRP)]
                for oj in range(QH):
                    wo_t = wpool.tile([128, NIT * 128], bf16, tag=wtag,
                                      name=f"wo{c}_{oj}")
                    nc.sync.dma_start(wo_t[:], wo.ap()[oj])
                    ps = pA.tile([128, CH], fp32, tag="pA",
                                 name=f"wops{c}_{oj}")
                    nmm = 0
                    for g, H in order:
                        cg, h = H // QH, H % QH
                        fo = (cg * 2 + (h % 2)) * CH
                        def mm(H=H, g=g, fo=fo, ps=ps, wo_t=wo_t,
                               nmm=nmm):
                            ensure_gt(g)
                            nc.tensor.matmul(
                                ps[:], wo_t[:, H * 128:(H + 1) * 128],
                                gts[g][:, fo:fo + CH],
                                start=(nmm == 0), stop=(nmm == NIT - 1),
                                skip_group_check=True)
                        yield mm
                        nmm += 1

                    def fin(ps=ps, oj=oj):
                        ot = out_p.tile([128, CH], fp32, tag="ot")
                        nc.vector.tensor_copy(ot[:], ps[:])
                        nc.scalar.dma_start(
                            outT.ap()[oj * 128:(oj + 1) * 128,
                                      c * CH:(c + 1) * CH], ot[:])
                    yield fin

            def attn(c, qt, wo_iter, xt_next=None, wo_start_h=1):
                blocks = plan[c]
                nb = len(blocks)
                # one wo matmul per attention block roughly balances PE
                # against the exp-bound ACT chain; the rest drains after
                wo_per_block = 1
                pending = None
                pending_gathers = []

                def gather(g):
                    nc.gpsimd.collective_compute(
                        "AllGather", mybir.AluOpType.bypass,
                        replica_groups=REPLICA_GROUPS,
                        ins=[bnc_in[c][g].ap().opt()],
                        outs=[bnc_out[c][g].ap().opt()])
                    if xt_next is not None:
                        issue_xt(xt_next[g], nc.gpsimd)
                for h in range(QH):
                    kvh = h // (QH // KVH)
                    qs = qt[:, h * CH:(h + 1) * CH]
                    po_t = po.tile([128, CH], fp32, tag="po")
                    acc = attn_p.tile([128, CH], fp32, tag="acc", bufs=2)
                    sc_tiles = {}
                    acc_init = [False, False]   # per q-half of acc

                    def emit_sc(bi):
                        t, _, qlo, qw = blocks[bi]
                        p = psc.tile([128, CH], fp32, tag="psc")
                        nc.tensor.matmul(
                            p[:, :qw], kT_sb[kvh][:, t * KB:(t + 1) * KB],
                            qs[:, qlo:qlo + qw],
                            start=True, stop=True, skip_group_check=True)
                        sc_tiles[bi] = p

                    emit_sc(0)
                    for bi in range(nb):
                        if bi + 1 < nb:
                            emit_sc(bi + 1)
                        if bi == 2 and pending is not None:
                            pending()
                            pending = None
                            for qd in pending_gathers:
                                gather(qd)
                            pending_gathers = []
                        t, mi, qlo, qw = blocks[bi]
                        p = sc_tiles.pop(bi)
                        pr = probs_p.tile([KB, CH], bf16, tag="pr")
                        nc.scalar.activation(pr[:, :qw], p[:, :qw], Exp,
                                             scale=SM_SCALE)
                        if mi is not None:
                            nc.vector.tensor_mul(
                                pr[:, :qw], pr[:, :qw],
                                dm_sb[:, mi * MW:mi * MW + qw])
                        first, last = (bi == 0), (bi == nb - 1)
                        vsl = v_sb[:, t * KVD + kvh * 128:t * KVD + (kvh + 1) * 128]
                        nc.tensor.matmul(po_t[:, qlo:qlo + qw], vsl,
                                         pr[:, :qw], start=first,
                                         stop=last, skip_group_check=True)
                        # acc init tracking: full-width blocks come first in
                        # every plan; half-width regions init on first touch
                        if qw == CH:
                            if bi == 0:
                                nc.vector.tensor_copy(acc[:], pr[:])
                                acc_init = [True, True]
                            else:
                                nc.vector.tensor_add(acc[:], acc[:], pr[:])
                        else:
                            hf = qlo // MW
                            asl = acc[:, qlo:qlo + qw]
                            if not acc_init[hf]:
                                nc.vector.tensor_copy(asl, pr[:, :qw])
                                acc_init[hf] = True
                            else:
                                nc.vector.tensor_add(asl, asl, pr[:, :qw])
                        if wo_iter is not None and h >= wo_start_h:
                            for _ in range(wo_per_block):
                                step = next(wo_iter, None)
                                if step is None:
                                    wo_iter = None
                                    break
                                step()
                    if pending is not None:
                        pending()
                        for qd in pending_gathers:
                            gather(qd)
                        pending_gathers = []
                    pending = make_tail(c, h, po_t, acc)
                    if h % 2 == 1:
                        pending_gathers.append(h // 2)
                pending()
                gather(NQD - 1)
                return wo_iter

            # pipeline: wo(c) interleaved into attn(c+1) block-by-block so the
            # all-gather hides and PE bubbles (exp-bound attention) fill up;
            # xt(c+1) DMAs issue at the head of attn(c) on the idle gpsimd ring
            qts = {}
            xtq = xtq0
            leftover = None
            for c in range(NCH):
                qts[c] = qt_p.tile([128, QH * CH], bf16, tag="qt", name=f"qt{c}")
                proj(c, qts[c], xtq)
                if debug_dumps and c == 0:
                    nc.sync.dma_start(dbg["qt0"].ap(), qts[c][:])
                # drain the previous chunk's un-interleaved wo remainder here,
                # after proj: by now its gathers are long done, so these MMs
                # never block the in-order PE stream on a collective
                if leftover is not None:
                    for step in leftover:
                        step()
                nxt = alloc_xt(c + 1) if c + 1 < NCH else None
                leftover = attn(c, qts.pop(c),
                                wo_steps(c - 1) if c >= 1 else None,
                                xt_next=nxt, wo_start_h=3 if c == 1 else 1)
                xtq = nxt
            if leftover is not None:
                for step in leftover:
                    step()
            for step in wo_steps(NCH - 1, wpool=qt_p, wtag="qt",
                                 gpool=xt_p, gtagf=lambda g: "xt",
                                 geng=nc.scalar):
                step()
            if debug_dumps:
                nc.sync.dma_start(dbg["kt0"].ap(), kT_sb[0][:])
                nc.sync.dma_start(dbg["v"].ap(), v_sb[:])

    nc.compile()
    return nc


# ---------------------------------------------------------------- host side
def _rope_cache():
    fi = np.arange(0, D, 2, dtype=np.float32)
    inv = 1.0 / 10000.0 ** (fi / D)
    ang = np.outer(np.arange(S, dtype=np.float32), inv)  # (S, 64)
    cos = np.concatenate([np.cos(ang)] * 2, -1)          # (S, 128)
    sin = np.sin(ang)
    sinS = np.concatenate([-sin, sin], -1)               # signed
    return (np.ascontiguousarray(cos.T).astype(BF16),
            np.ascontiguousarray(sinS.T).astype(BF16))


def _plan_from_mask(mask):
    """Returns (plan, dmask_per_batch[b] -> np[nm,128,MW] bf16).

    plan[c] = list of (t, mask_idx_or_None, q_offset, q_width) k-blocks for
    q-chunk c. Full-width (CH) unmasked blocks always precede half-width (MW)
    blocks — the kernel's acc-init logic relies on this. Causal diagonals are
    handled at half-width so the upper-left q-half skips its two empty
    k-blocks entirely (~25% less diagonal matmul work)."""
    m = np.asarray(mask[:, 0])                    # (B, S, S) bool, [q, k]
    tril = np.tril(np.ones((S, S), bool))
    if all(np.array_equal(m[b], tril) for b in range(B)):
        plan = []
        for c in range(NCH):
            blk = [(t, None, 0, CH) for t in range(4 * c)]
            blk += [(4 * c, 0, 0, MW), (4 * c + 1, 1, 0, MW),
                    (4 * c, None, MW, MW), (4 * c + 1, None, MW, MW),
                    (4 * c + 2, 0, MW, MW), (4 * c + 3, 1, MW, MW)]
            plan.append(blk)
        # mask 0: q >= k within the block; mask 1: q >= 128 + k
        dm = np.zeros((2, KB, MW), np.float32)
        for p in range(KB):
            dm[0, p, p:] = 1.0
            dm[1, p, 128 + p:] = 1.0
        dms = [dm.astype(BF16)] * B
        return plan, dms
    if m.all():
        plan = [[(t, None, 0, CH) for t in range(S // KB)]
                for _ in range(NCH)]
        z = np.zeros((1, KB, MW), BF16)
        return plan, [z] * B
    # generic: classify blocks against the union across batches; masked
    # blocks are split into q-halves (full first, partials after)
    mT = [np.ascontiguousarray(m[b].T) for b in range(B)]  # [k, q]
    plan, tiles = [], [[] for _ in range(B)]
    nm = 0
    for c in range(NCH):
        full, part = [], []
        for t in range(S // KB):
            subs = [mT[b][t * KB:(t + 1) * KB, c * CH:(c + 1) * CH]
                    for b in range(B)]
            if all(not s.any() for s in subs):
                continue
            if all(s.all() for s in subs):
                full.append((t, None, 0, CH))
                continue
            for hf in range(2):
                hsubs = [s[:, hf * MW:(hf + 1) * MW] for s in subs]
                if all(not hs.any() for hs in hsubs):
                    continue
                if all(hs.all() for hs in hsubs):
                    part.append((t, None, hf * MW, MW))
                else:
                    part.append((t, nm, hf * MW, MW))
                    for b in range(B):
                        tiles[b].append(hsubs[b].astype(BF16))
                    nm += 1
        plan.append(full + part)
    dms = [np.stack(tiles[b]) if nm else np.zeros((1, KB, MW), BF16)
           for b in range(B)]
    return plan, dms


def _pack_ip(w, nj):
    """[HID, nj*d] -> [nj, 128, NIT*d] tile-packed (i along free)."""
    hid, cols = w.shape
    d = cols // nj
    r = w.reshape(NIT, 128, nj, d).transpose(2, 1, 0, 3)
    return np.ascontiguousarray(r.reshape(nj, 128, NIT * d))


def _prep_inputs(x, mask, Wq, Wk, Wv, Wo):
    cosT, sinS = _rope_cache()
    plan, dms = _plan_from_mask(mask)
    dms_packed = []
    for b in range(B):
        dm = dms[b]  # [nm, 128, 512]
        dms_packed.append(np.ascontiguousarray(
            dm.transpose(1, 0, 2).reshape(KB, -1)))
    xp = {}
    for b in range(B):
        xb = np.ascontiguousarray(x[b].T).astype(BF16)      # [HID, S]
        r = xb.reshape(NIT, 128, NCH, CH).transpose(2, 1, 0, 3)
        xp[b] = np.ascontiguousarray(r.reshape(NCH, 128, NIT * CH))
    in_maps = []
    for c in range(NCORES):
        b, g = c // GRP, c % GRP
        wq_g = Wq[:, g * QD:(g + 1) * QD].astype(BF16)
        wk_g = Wk[:, g * KVD:(g + 1) * KVD].astype(BF16)
        wv_g = Wv[:, g * KVD:(g + 1) * KVD].astype(BF16)
        wo_g = Wo[:, g * QD:(g + 1) * QD].astype(BF16)
        in_maps.append({
            "xT": xp[b],
            "wq": _pack_ip(wq_g, QH),
            "wk": _pack_ip(wk_g, 1)[0],
            "wv": _pack_ip(wv_g, 1)[0],
            "wo": _pack_ip(wo_g, QH),
            "cosT": cosT,
            "sinS": sinS,
            "dmask": dms_packed[b],
        })
    return plan, in_maps


def _get_nc(plan, nmask, debug_dumps=False):
    key = (tuple(tuple(blk) for blk in plan), nmask, debug_dumps)
    if key not in _CACHE:
        _CACHE[key] = build_nc(plan, nmask, debug_dumps)
    return _CACHE[key]


def run(x, mask, Wq, Wk, Wv, Wo, trace=False, debug_dumps=False):
    plan, in_maps = _prep_inputs(x, mask, Wq, Wk, Wv, Wo)
    nmask = in_maps[0]["dmask"].shape[1] // MW
    nc = _get_nc(plan, nmask, debug_dumps)
    res = bass_utils.run_bass_kernel_spmd(
        nc, in_maps, core_ids=list(range(NCORES)), trace=trace)
    out = np.empty((B, S, HID), np.float32)
    for c in range(NCORES):
        b, g = c // GRP, c % GRP
        out[b, :, g * QD:(g + 1) * QD] = res.results[c]["outT"].T
    return out, res


def kernel(x, mask, Wq, Wk, Wv, Wo):
    # re-execute on a (rare) NaN/Inf flake: the compiled program is cached,
    # so a retry only re-runs the NEFF
    for _ in range(3):
        out, _ = run(np.asarray(x), np.asarray(mask), np.asarray(Wq),
                     np.asarray(Wk), np.asarray(Wv), np.asarray(Wo))
        if np.isfinite(out).all():
            break
    return out


# needed only when profiling (trace=True) inside this container
def install_ntff_hook():
    try:
        from antenv.axon_hooks import get_axon_ntff_profile_hook  # noqa: F401
        return
    except ImportError:
        pass
    import types
    import antenv
    try:
        from trn_agent_boot.trn_boot import _ntff_profile_via_ctypes
        hook = _ntff_profile_via_ctypes('/opt/axon/libaxon_pjrt.so')
    except Exception:
        hook = None
    mod = types.ModuleType("antenv.axon_hooks")
    state = {"h": hook}
    mod.get_axon_ntff_profile_hook = lambda: state["h"]
    mod.set_axon_ntff_profile_hook = lambda h: state.__setitem__("h", h)
    sys.modules["antenv.axon_hooks"] = mod
    antenv.axon_hooks = mod


install_ntff_hook()
bass_utils.upload_artifacts = lambda tmpdir: "local://" + str(tmpdir)



# revision 56
# speedup vs baseline: 1.0714x; 1.0714x over previous
"""Trainium2 Bass kernel for GQA attention (dense_transformer).

Sharding: 8 cores = 2-way data parallel (batch) x 4-way tensor parallel (heads).
Core c handles batch b=c//4, head group g=c%4 (8 q heads, 2 kv heads).
Per core: QKV projections (bf16, f32 accum), RoPE, causal attention with
transposed-scores layout (scoresT[k,q] -> probsT used directly as the moving
operand of the PV matmul; no on-chip transposes), per-chunk AllGather of
attention outputs within each 4-core group, then a column-sharded Wo matmul.
Host assembles disjoint output shards (pure unshard, no host math).

All inputs are host-packed into the exact SBUF tile layouts so every DMA is a
plain 2D transfer (contiguous per partition).
"""
import sys

sys.path.insert(0, "/opt/trn_rl_repo")

import numpy as np
import ml_dtypes

import concourse.bacc as bacc
import concourse.mybir as mybir
import concourse.tile as tile
from concourse import bass_isa
from concourse import bass_utils

BF16 = ml_dtypes.bfloat16

B, S, HID = 2, 2048, 4096
NH, NKV, D = 32, 8, 128
NCORES, GRP = 8, 4          # 2 groups of 4 cores
QH, KVH = NH // GRP, NKV // GRP   # 8 q heads, 2 kv heads per core
QD, KVD = QH * D, KVH * D         # 1024, 256
CH, NCH = 512, S // 512           # q-chunk size / count
KB = 128                          # k block
MW = CH // 2                      # masked-block q width (diag refinement)
NIT = HID // 128                  # 32 contraction tiles
SM_SCALE = float(D) ** -0.5
REPLICA_GROUPS = [[0, 1, 2, 3], [4, 5, 6, 7]]

_CACHE: dict = {}


# ---------------------------------------------------------------- builder
def build_nc(plan, nmask, debug_dumps=False):
    """plan[c] = list of (t, mask_idx_or_None) k-blocks for q-chunk c."""
    fp32, bf16, f32r = mybir.dt.float32, mybir.dt.bfloat16, mybir.dt.float32r
    nc = bacc.Bacc("TRN2", target_bir_lowering=False, debug=False,
                   num_devices=NCORES)

    # host-packed inputs (exact SBUF layouts; all DMAs contiguous/partition)
    xT = nc.dram_tensor("xT", [NCH, 128, NIT * CH], bf16, kind="ExternalInput")
    wq = nc.dram_tensor("wq", [QH, 128, NIT * 128], bf16, kind="ExternalInput")
    wk = nc.dram_tensor("wk", [128, NIT * KVD], bf16, kind="ExternalInput")
    wv = nc.dram_tensor("wv", [128, NIT * KVD], bf16, kind="ExternalInput")
    wo = nc.dram_tensor("wo", [QH, 128, NIT * 128], bf16, kind="ExternalInput")
    cosT = nc.dram_tensor("cosT", [D, S], bf16, kind="ExternalInput")
    sinS = nc.dram_tensor("sinS", [D, S], bf16, kind="ExternalInput")
    nm = max(nmask, 1)
    dmask = nc.dram_tensor("dmask", [KB, nm * MW], bf16, kind="ExternalInput")
    outT = nc.dram_tensor("outT", [QD, S], fp32, kind="ExternalOutput")

    # partition-major bounce buffers, one per head-quarter (2 heads): each
    # gather issues once its 2 heads are flushed; finer granularity loses to
    # per-op CC overhead, coarser exposes latency at chunk boundaries.
    # Gather concatenates the 4 group members along dim0 -> [GRP*128, 2*CH]
    NQD = QH // 2
    bnc_in = [[nc.dram_tensor(f"bnc_in{c}_{g}", [128, 2 * CH], bf16)
               for g in range(NQD)] for c in range(NCH)]
    bnc_out = [[nc.dram_tensor(f"bnc_out{c}_{g}", [GRP * 128, 2 * CH], bf16)
                for g in range(NQD)] for c in range(NCH)]

    # tiny buffers for a warm-up AllGather that absorbs the CC pipeline's
    # cold-start / peer-rendezvous cost (~12us) during proj(0)
    warm_in = nc.dram_tensor("warm_in", [128, 8], bf16)
    warm_out = nc.dram_tensor("warm_out", [GRP * 128, 8], bf16)
    import os as _os
    _salt = _os.environ.get("KSALT", "")
    if _salt:
        nc.dram_tensor(f"salt{_salt}", [1, 8], bf16)

    dbg = {}
    if debug_dumps:
        dbg["qt0"] = nc.dram_tensor("dbg_qt0", [128, QH * CH], bf16,
                                    kind="ExternalOutput")
        dbg["kt0"] = nc.dram_tensor("dbg_kt0", [D, S], bf16,
                                    kind="ExternalOutput")
        dbg["v"] = nc.dram_tensor("dbg_v", [128, (S // 128) * KVD], bf16,
                                  kind="ExternalOutput")

    with tile.TileContext(nc) as tc:
        from contextlib import ExitStack
        with ExitStack() as ctx:
            P = lambda **kw: ctx.enter_context(tc.tile_pool(**kw))
            const_p = P(name="const", bufs=1)
            wkv_p = P(name="wkv", bufs=1)
            res_p = P(name="res", bufs=1)         # kT, v, cos, sin, masks
            xt_p = P(name="xt", bufs=4)
            wq_p = P(name="wqp", bufs=3)
            qt_p = P(name="qt", bufs=2)
            rope_p = P(name="rope", bufs=2)
            probs_p = P(name="probs", bufs=3)
            attn_p = P(name="attn", bufs=3)
            gath_p = P(name="gath", bufs=1)
            wo_p = P(name="wop", bufs=2)
            out_p = P(name="outp", bufs=2)
            recip_p = P(name="recip", bufs=2)
            pA = P(name="pA", bufs=2, space="PSUM")
            psc = P(name="psc", bufs=3, space="PSUM")
            po = P(name="po", bufs=2, space="PSUM")
            psums = P(name="psums", bufs=1, space="PSUM")

            # constants
            ones_cf = const_p.tile([128, 1], fp32, tag="ones_cf")
            nc.gpsimd.memset(ones_cf[:], 1.0)
            ones_cr = const_p.tile([128, 1], f32r, tag="ones_cr")
            nc.vector.tensor_copy(ones_cr[:], ones_cf[:])

            # x^T chunk loads: 4 quarter tiles (8 i-tiles each). Chunk 0 goes
            # on the scalar HWDGE ring (fast, empty at t=0) so Q-proj can
            # start ~4us in; later chunks prefetch on the idle gpsimd ring,
            # one quarter after each gather trigger of the previous attn.
            NXQ = 4
            XQW = (NIT // NXQ) * CH   # free width of one quarter

            def alloc_xt(c):
                return [(xt_p.tile([128, XQW], bf16, tag="xt",
                                   name=f"xt{c}_{q}"), c, q)
                        for q in range(NXQ)]

            def issue_xt(entry, eng):
                t, c, q = entry
                eng.dma_start(t[:], xT.ap()[c][:, q * XQW:(q + 1) * XQW])

            def xt_sl(xtq, i, lo, hi):
                """slice of contraction tile i, columns [lo:hi) of the chunk"""
                q, r = divmod(i, NIT // NXQ)
                return xtq[q][0][:, r * CH + lo:r * CH + hi]

            xtq0 = alloc_xt(0)
            for e in xtq0:
                issue_xt(e, nc.scalar)

            # resident loads (all straight 2D); cos/sin first (needed by the
            # first rope ~12us in), then wk/wv (k-proj ~70us in), dmask last
            cos_sb = res_p.tile([D, S], bf16, tag="cos")
            nc.scalar.dma_start(cos_sb[:], cosT.ap())
            sin_sb = res_p.tile([D, S], bf16, tag="sin")
            nc.scalar.dma_start(sin_sb[:], sinS.ap())
            wk_sb = wkv_p.tile([128, NIT * KVD], bf16, tag="wk")
            nc.scalar.dma_start(wk_sb[:], wk.ap())
            wv_sb = wkv_p.tile([128, NIT * KVD], bf16, tag="wv")
            nc.scalar.dma_start(wv_sb[:], wv.ap())
            dm_sb = res_p.tile([KB, nm * MW], bf16, tag="dm")
            nc.scalar.dma_start(dm_sb[:], dmask.ap())
            kT_sb = [res_p.tile([D, S], bf16, tag=f"kT{kj}", name=f"kT{kj}")
                     for kj in range(KVH)]
            v_sb = res_p.tile([128, (S // 128) * KVD], bf16, tag="v")

            # warm up the collective pipeline (cold-start is ~12us) so chunk
            # 0's real gathers start promptly
            nc.gpsimd.collective_compute(
                "AllGather", mybir.AluOpType.bypass,
                replica_groups=REPLICA_GROUPS,
                ins=[warm_in.ap().opt()], outs=[warm_out.ap().opt()])

            Exp = mybir.ActivationFunctionType.Exp

            def rope(dst, ps, c):
                """dst (bf16 [128,CH]) = rope(ps) with cos/sin chunk c."""
                cs = cos_sb[:, c * CH:(c + 1) * CH]
                sn = sin_sb[:, c * CH:(c + 1) * CH]
                m1 = rope_p.tile([128, CH], fp32, tag="m1")
                m2 = rope_p.tile([128, CH], fp32, tag="m2")
                nc.vector.tensor_mul(m1[:], ps[:], cs)
                nc.vector.tensor_mul(m2[0:64, :], ps[64:128, :], sn[0:64, :])
                nc.vector.tensor_mul(m2[64:128, :], ps[0:64, :], sn[64:128, :])
                nc.vector.tensor_add(dst, m1[:], m2[:])

            def proj(c, qt, xtq):
                for j in range(QH):
                    wq_t = wq_p.tile([128, NIT * 128], bf16, tag="wq")
                    nc.sync.dma_start(wq_t[:], wq.ap()[j])
                    ps = pA.tile([128, CH], fp32, tag="pA")
                    for i in range(NIT):
                        nc.tensor.matmul(
                            ps[:], wq_t[:, i * 128:(i + 1) * 128],
                            xt_sl(xtq, i, 0, CH),
                            start=(i == 0), stop=(i == NIT - 1))
                    rope(qt[:, j * CH:(j + 1) * CH], ps, c)
                for kj in range(KVH):
                    ps = pA.tile([128, CH], fp32, tag="pA")
                    for i in range(NIT):
                        nc.tensor.matmul(
                            ps[:], wk_sb[:, i * KVD + kj * 128:
                                         i * KVD + (kj + 1) * 128],
                            xt_sl(xtq, i, 0, CH),
                            start=(i == 0), stop=(i == NIT - 1))
                    rope(kT_sb[kj][:, c * CH:(c + 1) * CH], ps, c)
                for t in range(CH // 128):
                    ps = pA.tile([128, KVD], fp32, tag="pA")
                    for i in range(NIT):
                        nc.tensor.matmul(
                            ps[:], xt_sl(xtq, i, t * 128, (t + 1) * 128),
                            wv_sb[:, i * KVD:(i + 1) * KVD],
                            start=(i == 0), stop=(i == NIT - 1))
                    sb = (c * (CH // 128) + t) * KVD
                    nc.scalar.copy(v_sb[:, sb:sb + KVD], ps[:])

            # deferred normalization tail: at head h's end the reciprocal of
            # the prob-sum is taken on DVE and broadcast across partitions on
            # the idle gpsimd engine; the final multiply + DMA (flush) run
            # while head h+1's scores are already streaming on PE.
            def make_tail(c, h, po_t, su_t):
                rc = recip_p.tile([1, CH], fp32, tag="rc", name=f"rc{c}_{h}",
                                  bufs=1)
                nc.vector.reciprocal_approx_fast(rc[:], su_t[:])
                bc = recip_p.tile([128, CH], fp32, tag="bc", name=f"bc{c}_{h}",
                                  bufs=1)
                nc.gpsimd.partition_broadcast(bc[:], rc[:], channels=128)

                def flush():
                    at = attn_p.tile([128, CH], bf16, tag="at")
                    nc.vector.tensor_mul(at[:], po_t[:], bc[:])
                    nc.scalar.dma_start(
                        bnc_in[c][h // 2].ap()[:, (h % 2) * CH:
                                               (h % 2 + 1) * CH], at[:])
                return flush

            def wo_steps(c, wpool=None, wtag="wo", gpool=None, gtagf=None,
                         geng=None):
                """Generator of emission closures for the Wo phase of chunk c
                (interleaved into the next chunk's attention to fill PE
                bubbles left by the exp chain). Contraction runs quarter-major
                so quarter-gather g gates only tiles 8g..8g+7 of each chain —
                the final chunk's last gather overlaps 3/4 of its Wo work.
                The final chunk passes retired pools (qt for weights, xt for
                gathers) and the scalar ring for gathers so its DMAs never
                queue behind the leftover drain's weight loads."""
                wpool = wpool or wo_p
                gpool = gpool or gath_p
                gtagf = gtagf or (lambda g: f"gt{g}")
                geng = geng or nc.sync
                gts = [None] * NQD

                def ensure_gt(g):
                    # lazily emitted right before the first consuming matmul:
                    # a not-yet-done collective then only delays weight
                    # prefetches, never the ACT exp stream
                    if gts[g] is None:
                        gt = gpool.tile([128, GRP * 2 * CH], bf16,
                                        tag=gtagf(g), name=f"gt{c}_{g}")
                        geng.dma_start(
                            gt[:].rearrange("p (g f) -> p g f", g=GRP),
                            bnc_out[c][g].ap().rearrange("(g p) f -> p g f",
                                                         p=128))
                        gts[g] = gt
                # hid tile H = cg*QH + h lives in quarter-gather h//2 at
                # column (cg*2 + h%2)*CH of that gather tile
                order = [(h // 2, cg * QH + h) for h in range(QH)
                         for cg in range(GRP)]
                for oj in range(QH):
                    wo_t = wpool.tile([128, NIT * 128], bf16, tag=wtag,
                                      name=f"wo{c}_{oj}")
                    nc.sync.dma_start(wo_t[:], wo.ap()[oj])
                    ps = pA.tile([128, CH], fp32, tag="pA",
                                 name=f"wops{c}_{oj}")
                    nmm = 0
                    for g, H in order:
                        cg, h = H // QH, H % QH
                        fo = (cg * 2 + (h % 2)) * CH
                        def mm(H=H, g=g, fo=fo, ps=ps, wo_t=wo_t,
                               nmm=nmm):
                            ensure_gt(g)
                            nc.tensor.matmul(
                                ps[:], wo_t[:, H * 128:(H + 1) * 128],
                                gts[g][:, fo:fo + CH],
                                start=(nmm == 0), stop=(nmm == NIT - 1),
                                skip_group_check=True)
                        yield mm
                        nmm += 1

                    def fin(ps=ps, oj=oj):
                        ot = out_p.tile([128, CH], fp32, tag="ot")
                        nc.vector.tensor_copy(ot[:], ps[:])
                        nc.scalar.dma_start(
                            outT.ap()[oj * 128:(oj + 1) * 128,
                                      c * CH:(c + 1) * CH], ot[:])
                    yield fin

            def attn(c, qt, wo_iter, xt_next=None, wo_start_h=1):
                blocks = plan[c]
                nb = len(blocks)
                # one wo matmul per attention block roughly balances PE
                # against the exp-bound ACT chain; the rest drains after
                wo_per_block = 1
                pending = None
                pending_gathers = []

                def gather(g):
                    nc.gpsimd.collective_compute(
                        "AllGather", mybir.AluOpType.bypass,
                        replica_groups=REPLICA_GROUPS,
                        ins=[bnc_in[c][g].ap().opt()],
                        outs=[bnc_out[c][g].ap().opt()])
                    if xt_next is not None:
                        issue_xt(xt_next[g], nc.gpsimd)
                for h in range(QH):
                    kvh = h // (QH // KVH)
                    qs = qt[:, h * CH:(h + 1) * CH]
                    po_t = po.tile([128, CH], fp32, tag="po")
                    su_t = psums.tile([1, CH], fp32, tag="su")
                    acc = attn_p.tile([128, CH], f32r, tag="acc", bufs=2)
                    sc_tiles = {}
                    acc_init = [False, False]   # per q-half of acc

                    def emit_sc(bi):
                        t, _, qlo, qw = blocks[bi]
                        p = psc.tile([128, CH], fp32, tag="psc")
                        nc.tensor.matmul(
                            p[:, :qw], kT_sb[kvh][:, t * KB:(t + 1) * KB],
                            qs[:, qlo:qlo + qw],
                            start=True, stop=True, skip_group_check=True)
                        sc_tiles[bi] = p

                    emit_sc(0)
                    for bi in range(nb):
                        if bi + 1 < nb:
                            emit_sc(bi + 1)
                        if bi == 2 and pending is not None:
                            pending()
                            pending = None
                            for qd in pending_gathers:
                                gather(qd)
                            pending_gathers = []
                        t, mi, qlo, qw = blocks[bi]
                        p = sc_tiles.pop(bi)
                        pr = probs_p.tile([KB, CH], bf16, tag="pr")
                        nc.scalar.activation(pr[:, :qw], p[:, :qw], Exp,
                                             scale=SM_SCALE)
                        if mi is not None:
                            nc.vector.tensor_mul(
                                pr[:, :qw], pr[:, :qw],
                                dm_sb[:, mi * MW:mi * MW + qw])
                        first, last = (bi == 0), (bi == nb - 1)
                        vsl = v_sb[:, t * KVD + kvh * 128:t * KVD + (kvh + 1) * 128]
                        nc.tensor.matmul(po_t[:, qlo:qlo + qw], vsl,
                                         pr[:, :qw], start=first,
                                         stop=last, skip_group_check=True)
                        # acc init tracking: full-width blocks come first in
                        # every plan; half-width regions init on first touch
                        if qw == CH:
                            if bi == 0:
                                nc.vector.tensor_copy(acc[:], pr[:])
                                acc_init = [True, True]
                            else:
                                nc.vector.tensor_add(acc[:], acc[:], pr[:])
                        else:
                            hf = qlo // MW
                            asl = acc[:, qlo:qlo + qw]
                            if not acc_init[hf]:
                                nc.vector.tensor_copy(asl, pr[:, :qw])
                                acc_init[hf] = True
                            else:
                                nc.vector.tensor_add(asl, asl, pr[:, :qw])
                        if wo_iter is not None and h >= wo_start_h:
                            for _ in range(wo_per_block):
                                step = next(wo_iter, None)
                                if step is None:
                                    wo_iter = None
                                    break
                                step()
                    # single ones-matmul on the accumulated probs
                    nc.tensor.matmul(su_t[:], ones_cr[:], acc[:], start=True,
                                     stop=True, skip_group_check=True)
                    if pending is not None:
                        pending()
                        for qd in pending_gathers:
                            gather(qd)
                        pending_gathers = []
                    pending = make_tail(c, h, po_t, su_t)
                    if h % 2 == 1:
                        pending_gathers.append(h // 2)
                pending()
                gather(NQD - 1)
                return wo_iter

            # pipeline: wo(c) interleaved into attn(c+1) block-by-block so the
            # all-gather hides and PE bubbles (exp-bound attention) fill up;
            # xt(c+1) DMAs issue at the head of attn(c) on the idle gpsimd ring
            qts = {}
            xtq = xtq0
            leftover = None
            for c in range(NCH):
                qts[c] = qt_p.tile([128, QH * CH], bf16, tag="qt", name=f"qt{c}")
                proj(c, qts[c], xtq)
                if debug_dumps and c == 0:
                    nc.sync.dma_start(dbg["qt0"].ap(), qts[c][:])
                # drain the previous chunk's un-interleaved wo remainder here,
                # after proj: by now its gathers are long done, so these MMs
                # never block the in-order PE stream on a collective
                if leftover is not None:
                    for step in leftover:
                        step()
                nxt = alloc_xt(c + 1) if c + 1 < NCH else None
                leftover = attn(c, qts.pop(c),
                                wo_steps(c - 1) if c >= 1 else None,
                                xt_next=nxt, wo_start_h=3 if c == 1 else 1)
                xtq = nxt
            if leftover is not None:
                for step in leftover:
                    step()
            for step in wo_steps(NCH - 1, wpool=qt_p, wtag="qt",
                                 gpool=xt_p, gtagf=lambda g: "xt",
                                 geng=nc.scalar):
                step()
            if debug_dumps:
                nc.sync.dma_start(dbg["kt0"].ap(), kT_sb[0][:])
                nc.sync.dma_start(dbg["v"].ap(), v_sb[:])

    nc.compile()
    return nc


# ---------------------------------------------------------------- host side
def _rope_cache():
    fi = np.arange(0, D, 2, dtype=np.float32)
    inv = 1.0 / 10000.0 ** (fi / D)
    ang = np.outer(np.arange(S, dtype=np.float32), inv)  # (S, 64)
    cos = np.concatenate([np.cos(ang)] * 2, -1)          # (S, 128)
    sin = np.sin(ang)
    sinS = np.concatenate([-sin, sin], -1)               # signed
    return (np.ascontiguousarray(cos.T).astype(BF16),
            np.ascontiguousarray(sinS.T).astype(BF16))


def _plan_from_mask(mask):
    """Returns (plan, dmask_per_batch[b] -> np[nm,128,MW] bf16).

    plan[c] = list of (t, mask_idx_or_None, q_offset, q_width) k-blocks for
    q-chunk c. Full-width (CH) unmasked blocks always precede half-width (MW)
    blocks — the kernel's acc-init logic relies on this. Causal diagonals are
    handled at half-width so the upper-left q-half skips its two empty
    k-blocks entirely (~25% less diagonal matmul work)."""
    m = np.asarray(mask[:, 0])                    # (B, S, S) bool, [q, k]
    tril = np.tril(np.ones((S, S), bool))
    if all(np.array_equal(m[b], tril) for b in range(B)):
        plan = []
        for c in range(NCH):
            blk = [(t, None, 0, CH) for t in range(4 * c)]
            blk += [(4 * c, 0, 0, MW), (4 * c + 1, 1, 0, MW),
                    (4 * c, None, MW, MW), (4 * c + 1, None, MW, MW),
                    (4 * c + 2, 0, MW, MW), (4 * c + 3, 1, MW, MW)]
            plan.append(blk)
        # mask 0: q >= k within the block; mask 1: q >= 128 + k
        dm = np.zeros((2, KB, MW), np.float32)
        for p in range(KB):
            dm[0, p, p:] = 1.0
            dm[1, p, 128 + p:] = 1.0
        dms = [dm.astype(BF16)] * B
        return plan, dms
    if m.all():
        plan = [[(t, None, 0, CH) for t in range(S // KB)]
                for _ in range(NCH)]
        z = np.zeros((1, KB, MW), BF16)
        return plan, [z] * B
    # generic: classify blocks against the union across batches; masked
    # blocks are split into q-halves (full first, partials after)
    mT = [np.ascontiguousarray(m[b].T) for b in range(B)]  # [k, q]
    plan, tiles = [], [[] for _ in range(B)]
    nm = 0
    for c in range(NCH):
        full, part = [], []
        for t in range(S // KB):
            subs = [mT[b][t * KB:(t + 1) * KB, c * CH:(c + 1) * CH]
                    for b in range(B)]
            if all(not s.any() for s in subs):
                continue
            if all(s.all() for s in subs):
                full.append((t, None, 0, CH))
                continue
            for hf in range(2):
                hsubs = [s[:, hf * MW:(hf + 1) * MW] for s in subs]
                if all(not hs.any() for hs in hsubs):
                    continue
                if all(hs.all() for hs in hsubs):
                    part.append((t, None, hf * MW, MW))
                else:
                    part.append((t, nm, hf * MW, MW))
                    for b in range(B):
                        tiles[b].append(hsubs[b].astype(BF16))
                    nm += 1
        plan.append(full + part)
    dms = [np.stack(tiles[b]) if nm else np.zeros((1, KB, MW), BF16)
           for b in range(B)]
    return plan, dms


def _pack_ip(w, nj):
    """[HID, nj*d] -> [nj, 128, NIT*d] tile-packed (i along free)."""
    hid, cols = w.shape
    d = cols // nj
    r = w.reshape(NIT, 128, nj, d).transpose(2, 1, 0, 3)
    return np.ascontiguousarray(r.reshape(nj, 128, NIT * d))


def _prep_inputs(x, mask, Wq, Wk, Wv, Wo):
    cosT, sinS = _rope_cache()
    plan, dms = _plan_from_mask(mask)
    dms_packed = []
    for b in range(B):
        dm = dms[b]  # [nm, 128, 512]
        dms_packed.append(np.ascontiguousarray(
            dm.transpose(1, 0, 2).reshape(KB, -1)))
    xp = {}
    for b in range(B):
        xb = np.ascontiguousarray(x[b].T).astype(BF16)      # [HID, S]
        r = xb.reshape(NIT, 128, NCH, CH).transpose(2, 1, 0, 3)
        xp[b] = np.ascontiguousarray(r.reshape(NCH, 128, NIT * CH))
    in_maps = []
    for c in range(NCORES):
        b, g = c // GRP, c % GRP
        wq_g = Wq[:, g * QD:(g + 1) * QD].astype(BF16)
        wk_g = Wk[:, g * KVD:(g + 1) * KVD].astype(BF16)
        wv_g = Wv[:, g * KVD:(g + 1) * KVD].astype(BF16)
        wo_g = Wo[:, g * QD:(g + 1) * QD].astype(BF16)
        in_maps.append({
            "xT": xp[b],
            "wq": _pack_ip(wq_g, QH),
            "wk": _pack_ip(wk_g, 1)[0],
            "wv": _pack_ip(wv_g, 1)[0],
            "wo": _pack_ip(wo_g, QH),
            "cosT": cosT,
            "sinS": sinS,
            "dmask": dms_packed[b],
        })
    return plan, in_maps


def _get_nc(plan, nmask, debug_dumps=False):
    key = (tuple(tuple(blk) for blk in plan), nmask, debug_dumps)
    if key not in _CACHE:
        _CACHE[key] = build_nc(plan, nmask, debug_dumps)
    return _CACHE[key]


def run(x, mask, Wq, Wk, Wv, Wo, trace=False, debug_dumps=False):
    plan, in_maps = _prep_inputs(x, mask, Wq, Wk, Wv, Wo)
    nmask = in_maps[0]["dmask"].shape[1] // MW
    nc = _get_nc(plan, nmask, debug_dumps)
    res = bass_utils.run_bass_kernel_spmd(
        nc, in_maps, core_ids=list(range(NCORES)), trace=trace)
    out = np.empty((B, S, HID), np.float32)
    for c in range(NCORES):
        b, g = c // GRP, c % GRP
        out[b, :, g * QD:(g + 1) * QD] = res.results[c]["outT"].T
    return out, res


def kernel(x, mask, Wq, Wk, Wv, Wo):
    # re-execute on a (rare) NaN/Inf flake: the compiled program is cached,
    # so a retry only re-runs the NEFF
    for _ in range(3):
        out, _ = run(np.asarray(x), np.asarray(mask), np.asarray(Wq),
                     np.asarray(Wk), np.asarray(Wv), np.asarray(Wo))
        if np.isfinite(out).all():
            break
    return out


# needed only when profiling (trace=True) inside this container
def install_ntff_hook():
    try:
        from antenv.axon_hooks import get_axon_ntff_profile_hook  # noqa: F401
        return
    except ImportError:
        pass
    import types
    import antenv
    try:
        from trn_agent_boot.trn_boot import _ntff_profile_via_ctypes
        hook = _ntff_profile_via_ctypes('/opt/axon/libaxon_pjrt.so')
    except Exception:
        hook = None
    mod = types.ModuleType("antenv.axon_hooks")
    state = {"h": hook}
    mod.get_axon_ntff_profile_hook = lambda: state["h"]
    mod.set_axon_ntff_profile_hook = lambda h: state.__setitem__("h", h)
    sys.modules["antenv.axon_hooks"] = mod
    antenv.axon_hooks = mod


install_ntff_hook()
bass_utils.upload_artifacts = lambda tmpdir: "local://" + str(tmpdir)

